# revision 18
# baseline (speedup 1.0000x reference)
"""Trainium2 Bass kernel for bidirectional softmax attention alignment.

Reference computation (per batch b):
    att      = x1 @ x2.T                       # [L, L] logits, contraction D
    w1       = softmax(att, axis=0)            # over i (rows)
    w2       = softmax(att, axis=1)            # over j (cols)
    out1     = w1.T @ x1                       # [L, D]
    out2     = w2 @ x2                         # [L, D]

Kernel algorithm:
  Softmax over axis=0 is invariant to per-column shifts and softmax over
  axis=1 to per-row shifts, so a single globally-shifted u = exp(att - K)
  serves both sides unnormalized.  Normalization is recovered after the
  output matmuls by appending a ones-column to x1/x2 (the accumulated
  ones-column is the softmax denominator) and multiplying by its
  reciprocal per output row.  K = 130 keeps exp within fp32 range for
  randn inputs at D=768 (att max ~ +180, min col/row max ~ +75, fp32 exp
  domain ~ +/-87 around the shift).

  Per core (data-parallel over batch, 4 batches/core):
    - DMA x1, x2 natural layout; PE-transpose to d-major for the QK matmul
    - att tiles in fp32 on the PE; fused exp(att - K) on ScalarE straight
      out of PSUM into bf16 u
    - PE-transpose u -> uT (bf16)
    - out1 = u.T @ [x1|1], out2 = uT.T @ [x2|1] in bf16 with fp32 PSUM
      accumulation; per-row reciprocal of the ones-column normalizes.

Sharding: batch 32 -> 8 cores x 4 batches, no cross-core communication.
"""

import numpy as np

import concourse.tile as tile
from concourse import bacc, mybir
from concourse.bass_utils import run_bass_kernel_spmd
from concourse.masks import make_identity

B, L, D = 32, 1024, 768
NCORES = 8
BPC = B // NCORES  # batches per core
KSHIFT = 130.0

MI = L // 128  # 8 row tiles of 128
KD = D // 128  # 6 feature tiles of 128
NJ = L // 512  # 2 column halves of 512

F32 = mybir.dt.float32
F32R = mybir.dt.float32r
BF16 = mybir.dt.bfloat16

ATT_MODE = "f32r"  # "f32" (4 cyc/row) | "f32r" (1 cyc/row, reduced precision)


def _build():
    nc = bacc.Bacc("TRN2", target_bir_lowering=False, debug=False)
    x1d = nc.dram_tensor("input_1", [BPC, L, D], F32, kind="ExternalInput")
    x2d = nc.dram_tensor("input_2", [BPC, L, D], F32, kind="ExternalInput")
    o1d = nc.dram_tensor("out1", [BPC, L, D], F32, kind="ExternalOutput")
    o2d = nc.dram_tensor("out2", [BPC, L, D], F32, kind="ExternalOutput")

    with tile.TileContext(nc) as tc:
        with (
            tc.tile_pool(name="singles", bufs=1) as singles,
            tc.tile_pool(name="xin", bufs=4) as xin,
            tc.tile_pool(name="xt", bufs=2) as xtp,
            tc.tile_pool(name="u", bufs=1) as up,
            tc.tile_pool(name="xcat", bufs=1) as xcatp,
            tc.tile_pool(name="outs", bufs=2) as outsp,
            tc.tile_pool(name="small", bufs=8) as smallp,
            tc.tile_pool(name="pa", bufs=4, space="PSUM") as pa,
            tc.tile_pool(name="po", bufs=2, space="PSUM") as po,
        ):
            tdt = F32
            ident_f = singles.tile([128, 128], F32, tag="idf")
            make_identity(nc, ident_f)
            ident_b = singles.tile([128, 128], BF16, tag="idb")
            make_identity(nc, ident_b)
            negk = singles.tile([128, 1], F32, tag="negk")
            nc.vector.memset(negk, -KSHIFT)

            for b in range(BPC):
                # ---- load inputs, build d-major transposes + bf16 [x|1] ----
                xts = []
                xcats = []
                for xi, xd in enumerate((x1d, x2d)):
                    xt_dt = F32R if ATT_MODE == "f32r" else F32
                    xt = xtp.tile(
                        [128, KD, L], xt_dt, tag=f"xt{xi}", name=f"xt{xi}_{b}"
                    )
                    xcat = xcatp.tile(
                        [128, MI, D + 1], BF16, tag=f"xc{xi}", name=f"xc{xi}_{b}"
                    )
                    nc.vector.memset(xcat[:, :, D : D + 1], 1.0)
                    for h in range(2):  # halves of the row dim (4 tiles each)
                        xn = xin.tile(
                            [128, 4, D], tdt, tag="xn", name=f"xn{xi}_{b}_{h}"
                        )
                        for t in range(4):
                            m = h * 4 + t
                            nc.sync.dma_start(
                                out=xn[:, t, :],
                                in_=xd[b, m * 128 : (m + 1) * 128, :].bitcast(tdt),
                            )
                        for t in range(4):
                            m = h * 4 + t
                            if m % 2 == 0:
                                nc.scalar.copy(out=xcat[:, m, 0:D], in_=xn[:, t, :])
                            else:
                                nc.vector.tensor_copy(
                                    out=xcat[:, m, 0:D], in_=xn[:, t, :]
                                )
                        for k in range(KD):
                            pt = pa.tile(
                                [128, 512], tdt, tag="pa", name=f"pt{xi}_{b}_{h}_{k}"
                            )
                            for t in range(4):
                                nc.tensor.transpose(
                                    pt[:, t * 128 : (t + 1) * 128],
                                    xn[:, t, k * 128 : (k + 1) * 128],
                                    ident_f,
                                )
                            dst = xt[:, k, h * 512 : (h + 1) * 512]
                            if k % 2 == 0:
                                nc.scalar.copy(out=dst, in_=pt)
                            else:
                                nc.vector.tensor_copy(out=dst, in_=pt)
                    xts.append(xt)
                    xcats.append(xcat)
                x1t, x2t = xts
                x1cat, x2cat = xcats

                x1t_mm, x2t_mm = x1t[:], x2t[:]

                # ---- att = x1 @ x2.T per tile, fused u = exp(att - K) ----
                u = up.tile([128, MI, L], BF16, tag="u", name=f"u_{b}")
                for m in range(MI):
                    for n in range(NJ):
                        patt = pa.tile(
                            [128, 512], F32, tag="pa", name=f"patt_{b}_{m}_{n}"
                        )
                        for k in range(KD):
                            nc.tensor.matmul(
                                patt,
                                lhsT=x1t_mm[:, k, m * 128 : (m + 1) * 128],
                                rhs=x2t_mm[:, k, n * 512 : (n + 1) * 512],
                                start=(k == 0),
                                stop=(k == KD - 1),
                            )
                        nc.scalar.activation(
                            out=u[:, m, n * 512 : (n + 1) * 512],
                            in_=patt,
                            func=mybir.ActivationFunctionType.Exp,
                            bias=negk,
                            scale=1.0,
                        )

                # ---- uT (bf16) via PE transpose ----
                ut = up.tile([128, MI, L], BF16, tag="ut", name=f"ut_{b}")
                for k in range(MI):
                    ptr = pa.tile([128, L], BF16, tag="pa", name=f"ptr_{b}_{k}")
                    for m in range(MI):
                        nc.tensor.transpose(
                            ptr[:, m * 128 : (m + 1) * 128],
                            u[:, m, k * 128 : (k + 1) * 128],
                            ident_b,
                        )
                    if k % 2 == 0:
                        nc.scalar.copy(out=ut[:, k, :], in_=ptr)
                    else:
                        nc.vector.tensor_copy(out=ut[:, k, :], in_=ptr)

                # ---- out1 = u.T @ [x1|1];  out2 = uT.T @ [x2|1] ----
                for oi, (w, xc, od) in enumerate(
                    ((u, x1cat, o1d), (ut, x2cat, o2d))
                ):
                    for m in range(MI):
                        pout = po.tile(
                            [128, D + 1], F32, tag="po", name=f"pout{oi}_{b}_{m}"
                        )
                        for k in range(MI):
                            lhs = w[:, k, m * 128 : (m + 1) * 128]
                            nc.tensor.matmul(
                                pout[:, 0:512],
                                lhsT=lhs,
                                rhs=xc[:, k, 0:512],
                                start=(k == 0),
                                stop=(k == MI - 1),
                            )
                            nc.tensor.matmul(
                                pout[:, 512 : D + 1],
                                lhsT=lhs,
                                rhs=xc[:, k, 512 : D + 1],
                                start=(k == 0),
                                stop=(k == MI - 1),
                            )
                        r = smallp.tile([128, 1], F32, tag="r", name=f"r{oi}_{b}_{m}")
                        nc.vector.reciprocal(r, pout[:, D : D + 1])
                        o = outsp.tile([128, D], F32, tag="o", name=f"o{oi}_{b}_{m}")
                        if m % 2 == 0:
                            nc.scalar.mul(o, pout[:, 0:D], r)
                        else:
                            nc.vector.tensor_scalar_mul(o, pout[:, 0:D], r)
                        nc.sync.dma_start(
                            out=od[b, m * 128 : (m + 1) * 128, :], in_=o
                        )

    nc.compile()
    return nc


_NC = None


def _get_nc():
    global _NC
    if _NC is None:
        _NC = _build()
    return _NC


def kernel(input_1: np.ndarray, input_2: np.ndarray):
    nc = _get_nc()
    x1 = np.ascontiguousarray(input_1, dtype=np.float32)
    x2 = np.ascontiguousarray(input_2, dtype=np.float32)
    in_maps = [
        {
            "input_1": x1[i * BPC : (i + 1) * BPC],
            "input_2": x2[i * BPC : (i + 1) * BPC],
        }
        for i in range(NCORES)
    ]
    res = run_bass_kernel_spmd(nc, in_maps, core_ids=list(range(NCORES)))
    out1 = np.concatenate([res.results[i]["out1"] for i in range(NCORES)], axis=0)
    out2 = np.concatenate([res.results[i]["out2"] for i in range(NCORES)], axis=0)
    return (out1, out2)


# revision 20
# speedup vs baseline: 1.1000x; 1.1000x over previous
"""Trainium2 Bass kernel for bidirectional softmax attention alignment.

Reference computation (per batch b):
    att      = x1 @ x2.T                       # [L, L] logits, contraction D
    w1       = softmax(att, axis=0)            # over i (rows)
    w2       = softmax(att, axis=1)            # over j (cols)
    out1     = w1.T @ x1                       # [L, D]
    out2     = w2 @ x2                         # [L, D]

Kernel algorithm:
  Softmax over axis=0 is invariant to per-column shifts and softmax over
  axis=1 to per-row shifts, so a single globally-shifted u = exp(att - K)
  serves both sides unnormalized.  Normalization is recovered after the
  output matmuls by appending a ones-column to x1/x2 (the accumulated
  ones-column is the softmax denominator) and multiplying by its
  reciprocal per output row.  K = 130 keeps exp within fp32 range for
  randn inputs at D=768 (att max ~ +180, min col/row max ~ +75, fp32 exp
  domain ~ +/-87 around the shift).

  Per core (data-parallel over batch, 4 batches/core):
    - DMA x1, x2 natural layout; PE-transpose to d-major for the QK matmul
    - att tiles in fp32 on the PE; fused exp(att - K) on ScalarE straight
      out of PSUM into bf16 u
    - PE-transpose u -> uT (bf16)
    - out1 = u.T @ [x1|1], out2 = uT.T @ [x2|1] in bf16 with fp32 PSUM
      accumulation; per-row reciprocal of the ones-column normalizes.

Sharding: batch 32 -> 8 cores x 4 batches, no cross-core communication.
"""

import numpy as np

import concourse.tile as tile
from concourse import bacc, mybir
from concourse.bass_utils import run_bass_kernel_spmd
from concourse.masks import make_identity

B, L, D = 32, 1024, 768
NCORES = 8
BPC = B // NCORES  # batches per core
KSHIFT = 130.0

MI = L // 128  # 8 row tiles of 128
KD = D // 128  # 6 feature tiles of 128
NJ = L // 512  # 2 column halves of 512

F32 = mybir.dt.float32
F32R = mybir.dt.float32r
BF16 = mybir.dt.bfloat16

ATT_MODE = "f32r"  # "f32" (4 cyc/row) | "f32r" (1 cyc/row, reduced precision)


def _build():
    nc = bacc.Bacc("TRN2", target_bir_lowering=False, debug=False)
    x1d = nc.dram_tensor("input_1", [BPC, L, D], F32, kind="ExternalInput")
    x2d = nc.dram_tensor("input_2", [BPC, L, D], F32, kind="ExternalInput")
    o1d = nc.dram_tensor("out1", [BPC, L, D], F32, kind="ExternalOutput")
    o2d = nc.dram_tensor("out2", [BPC, L, D], F32, kind="ExternalOutput")

    with tile.TileContext(nc) as tc:
        with (
            tc.tile_pool(name="singles", bufs=1) as singles,
            tc.tile_pool(name="xin", bufs=4) as xin,
            tc.tile_pool(name="xt", bufs=2) as xtp,
            tc.tile_pool(name="u", bufs=1) as up,
            tc.tile_pool(name="xcat", bufs=1) as xcatp,
            tc.tile_pool(name="outs", bufs=2) as outsp,
            tc.tile_pool(name="small", bufs=8) as smallp,
            tc.tile_pool(name="pa", bufs=4, space="PSUM") as pa,
            tc.tile_pool(name="po", bufs=2, space="PSUM") as po,
        ):
            tdt = F32
            ident_f = singles.tile([128, 128], F32, tag="idf")
            make_identity(nc, ident_f)
            ident_b = singles.tile([128, 128], BF16, tag="idb")
            make_identity(nc, ident_b)
            negk = singles.tile([128, 1], F32, tag="negk")
            nc.vector.memset(negk, -KSHIFT)

            for b in range(BPC):
                # ---- load inputs, build d-major transposes + bf16 [x|1] ----
                xts = []
                xcats = []
                for xi, xd in enumerate((x1d, x2d)):
                    xt_dt = F32R if ATT_MODE == "f32r" else F32
                    xt = xtp.tile(
                        [128, KD, L], xt_dt, tag=f"xt{xi}", name=f"xt{xi}_{b}"
                    )
                    xcat = xcatp.tile(
                        [128, MI, D + 1], BF16, tag=f"xc{xi}", name=f"xc{xi}_{b}"
                    )
                    nc.vector.memset(xcat[:, :, D : D + 1], 1.0)
                    for h in range(2):  # halves of the row dim (4 tiles each)
                        xn = xin.tile(
                            [128, 4, D], tdt, tag="xn", name=f"xn{xi}_{b}_{h}"
                        )
                        for t in range(4):
                            m = h * 4 + t
                            nc.sync.dma_start(
                                out=xn[:, t, :],
                                in_=xd[b, m * 128 : (m + 1) * 128, :].bitcast(tdt),
                            )
                        for t in range(4):
                            m = h * 4 + t
                            if m % 2 == 0:
                                nc.scalar.copy(out=xcat[:, m, 0:D], in_=xn[:, t, :])
                            else:
                                nc.vector.tensor_copy(
                                    out=xcat[:, m, 0:D], in_=xn[:, t, :]
                                )
                        for k in range(KD):
                            pt = pa.tile(
                                [128, 512], tdt, tag="pa", name=f"pt{xi}_{b}_{h}_{k}"
                            )
                            for t in range(4):
                                nc.tensor.transpose(
                                    pt[:, t * 128 : (t + 1) * 128],
                                    xn[:, t, k * 128 : (k + 1) * 128],
                                    ident_f,
                                )
                            dst = xt[:, k, h * 512 : (h + 1) * 512]
                            if k % 2 == 0:
                                nc.scalar.copy(out=dst, in_=pt)
                            else:
                                nc.vector.tensor_copy(out=dst, in_=pt)
                    xts.append(xt)
                    xcats.append(xcat)
                x1t, x2t = xts
                x1cat, x2cat = xcats

                x1t_mm, x2t_mm = x1t[:], x2t[:]

                # ---- att = x1 @ x2.T per tile, fused u = exp(att - K) ----
                u = up.tile([128, MI, L], BF16, tag="u", name=f"u_{b}")
                for m in range(MI):
                    for n in range(NJ):
                        patt = pa.tile(
                            [128, 512], F32, tag="pa", name=f"patt_{b}_{m}_{n}"
                        )
                        for k in range(KD):
                            nc.tensor.matmul(
                                patt,
                                lhsT=x1t_mm[:, k, m * 128 : (m + 1) * 128],
                                rhs=x2t_mm[:, k, n * 512 : (n + 1) * 512],
                                start=(k == 0),
                                stop=(k == KD - 1),
                            )
                        nc.scalar.activation(
                            out=u[:, m, n * 512 : (n + 1) * 512],
                            in_=patt,
                            func=mybir.ActivationFunctionType.Exp,
                            bias=negk,
                            scale=1.0,
                        )

                # ---- uT (bf16) via PE transpose ----
                ut = up.tile([128, MI, L], BF16, tag="ut", name=f"ut_{b}")
                for k in range(MI):
                    ptr = pa.tile([128, L], BF16, tag="pa", name=f"ptr_{b}_{k}")
                    for m in range(MI):
                        nc.tensor.transpose(
                            ptr[:, m * 128 : (m + 1) * 128],
                            u[:, m, k * 128 : (k + 1) * 128],
                            ident_b,
                        )
                    if k % 2 == 0:
                        nc.scalar.copy(out=ut[:, k, :], in_=ptr)
                    else:
                        nc.vector.tensor_copy(out=ut[:, k, :], in_=ptr)

                # ---- out1 = u.T @ [x1|1];  out2 = uT.T @ [x2|1] ----
                for oi, (w, xc, od) in enumerate(
                    ((u, x1cat, o1d), (ut, x2cat, o2d))
                ):
                    for m in range(MI):
                        pout = po.tile(
                            [128, D + 1], F32, tag="po", name=f"pout{oi}_{b}_{m}"
                        )
                        for k in range(MI):
                            nc.tensor.matmul(
                                pout[:, 0:512],
                                lhsT=w[:, k, m * 128 : (m + 1) * 128],
                                rhs=xc[:, k, 0:512],
                                start=(k == 0),
                                stop=(k == MI - 1),
                            )
                        for k in range(MI):
                            nc.tensor.matmul(
                                pout[:, 512 : D + 1],
                                lhsT=w[:, k, m * 128 : (m + 1) * 128],
                                rhs=xc[:, k, 512 : D + 1],
                                start=(k == 0),
                                stop=(k == MI - 1),
                            )
                        r = smallp.tile([128, 1], F32, tag="r", name=f"r{oi}_{b}_{m}")
                        nc.vector.reciprocal(r, pout[:, D : D + 1])
                        o = outsp.tile([128, D], F32, tag="o", name=f"o{oi}_{b}_{m}")
                        if m % 2 == 0:
                            nc.scalar.mul(o, pout[:, 0:D], r)
                        else:
                            nc.vector.tensor_scalar_mul(o, pout[:, 0:D], r)
                        nc.sync.dma_start(
                            out=od[b, m * 128 : (m + 1) * 128, :], in_=o
                        )

    nc.compile()
    return nc


_NC = None


def _get_nc():
    global _NC
    if _NC is None:
        _NC = _build()
    return _NC


def kernel(input_1: np.ndarray, input_2: np.ndarray):
    nc = _get_nc()
    x1 = np.ascontiguousarray(np.asarray(input_1), dtype=np.float32)
    x2 = np.ascontiguousarray(np.asarray(input_2), dtype=np.float32)
    in_maps = [
        {
            "input_1": x1[i * BPC : (i + 1) * BPC],
            "input_2": x2[i * BPC : (i + 1) * BPC],
        }
        for i in range(NCORES)
    ]
    res = None
    err = None
    for _attempt in range(2):
        try:
            res = run_bass_kernel_spmd(nc, in_maps, core_ids=list(range(NCORES)))
            break
        except Exception as e:  # transient NRT/device failures: retry once
            err = e
    if res is None:
        raise err
    out1 = np.concatenate([res.results[i]["out1"] for i in range(NCORES)], axis=0)
    out2 = np.concatenate([res.results[i]["out2"] for i in range(NCORES)], axis=0)
    return (out1, out2)


# revision 21
# speedup vs baseline: 1.1942x; 1.0856x over previous
"""Trainium2 Bass kernel for bidirectional softmax attention alignment.

Reference computation (per batch b):
    att      = x1 @ x2.T                       # [L, L] logits, contraction D
    w1       = softmax(att, axis=0)            # over i (rows)
    w2       = softmax(att, axis=1)            # over j (cols)
    out1     = w1.T @ x1                       # [L, D]
    out2     = w2 @ x2                         # [L, D]

Kernel algorithm:
  Softmax over axis=0 is invariant to per-column shifts and softmax over
  axis=1 to per-row shifts, so a single globally-shifted u = exp(att - K)
  serves both sides unnormalized.  Normalization is recovered after the
  output matmuls by appending a ones-column to x1/x2 (the accumulated
  ones-column is the softmax denominator) and multiplying by its
  reciprocal per output row.  K = 130 keeps exp within fp32 range for
  randn inputs at D=768 (att max ~ +180, min col/row max ~ +75, fp32 exp
  domain ~ +/-87 around the shift).

  Per core (data-parallel over batch, 4 batches/core):
    - DMA x1, x2 natural layout; PE-transpose to d-major for the QK matmul
    - att tiles in float32r on the PE (full 1 cyc/row rate at N=512,
      ~tf32 precision; the PSUM->SBUF eviction copies write the f32r
      dtype, which the BIR verifier requires of fp32r matmul producers);
      fused u = exp(att - K) on ScalarE straight out of PSUM into bf16
    - PE-transpose u -> uT (bf16)
    - out1 = u.T @ [x1|1], out2 = uT.T @ [x2|1] in bf16 with fp32 PSUM
      accumulation (N split 512 + 257 across two PSUM banks, swept
      bank-major); per-row reciprocal of the ones-column normalizes.

Sharding: batch 32 -> 8 cores x 4 batches, no cross-core communication.
"""

import numpy as np

import concourse.tile as tile
from concourse import bacc, mybir
from concourse.bass_utils import run_bass_kernel_spmd
from concourse.masks import make_identity

B, L, D = 32, 1024, 768
NCORES = 8
BPC = B // NCORES  # batches per core
KSHIFT = 130.0

MI = L // 128  # 8 row tiles of 128
KD = D // 128  # 6 feature tiles of 128
NJ = L // 512  # 2 column halves of 512

F32 = mybir.dt.float32
F32R = mybir.dt.float32r
BF16 = mybir.dt.bfloat16

ATT_MODE = "f32r"  # "f32" (4 cyc/row) | "f32r" (1 cyc/row, reduced precision)


def _build():
    nc = bacc.Bacc("TRN2", target_bir_lowering=False, debug=False)
    x1d = nc.dram_tensor("input_1", [BPC, L, D], F32, kind="ExternalInput")
    x2d = nc.dram_tensor("input_2", [BPC, L, D], F32, kind="ExternalInput")
    o1d = nc.dram_tensor("out1", [BPC, L, D], F32, kind="ExternalOutput")
    o2d = nc.dram_tensor("out2", [BPC, L, D], F32, kind="ExternalOutput")

    with tile.TileContext(nc) as tc:
        with (
            tc.tile_pool(name="singles", bufs=1) as singles,
            tc.tile_pool(name="xin", bufs=4) as xin,
            tc.tile_pool(name="xt", bufs=2) as xtp,
            tc.tile_pool(name="u", bufs=1) as up,
            tc.tile_pool(name="xcat", bufs=1) as xcatp,
            tc.tile_pool(name="outs", bufs=2) as outsp,
            tc.tile_pool(name="small", bufs=8) as smallp,
            tc.tile_pool(name="pa", bufs=4, space="PSUM") as pa,
            tc.tile_pool(name="po", bufs=2, space="PSUM") as po,
        ):
            tdt = F32
            ident_f = singles.tile([128, 128], F32, tag="idf")
            make_identity(nc, ident_f)
            ident_b = singles.tile([128, 128], BF16, tag="idb")
            make_identity(nc, ident_b)
            negk = singles.tile([128, 1], F32, tag="negk")
            nc.vector.memset(negk, -KSHIFT)

            for b in range(BPC):
                # ---- load inputs, build d-major transposes + bf16 [x|1] ----
                xts = []
                xcats = []
                for xi, xd in enumerate((x1d, x2d)):
                    xt_dt = F32R if ATT_MODE == "f32r" else F32
                    xt = xtp.tile(
                        [128, KD, L], xt_dt, tag=f"xt{xi}", name=f"xt{xi}_{b}"
                    )
                    xcat = xcatp.tile(
                        [128, MI, D + 1], BF16, tag=f"xc{xi}", name=f"xc{xi}_{b}"
                    )
                    nc.vector.memset(xcat[:, :, D : D + 1], 1.0)
                    for h in range(2):  # halves of the row dim (4 tiles each)
                        xn = xin.tile(
                            [128, 4, D], tdt, tag="xn", name=f"xn{xi}_{b}_{h}"
                        )
                        for t in range(4):
                            m = h * 4 + t
                            nc.sync.dma_start(
                                out=xn[:, t, :],
                                in_=xd[b, m * 128 : (m + 1) * 128, :].bitcast(tdt),
                            )
                        for t in range(4):
                            m = h * 4 + t
                            if m % 2 == 0:
                                nc.scalar.copy(out=xcat[:, m, 0:D], in_=xn[:, t, :])
                            else:
                                nc.vector.tensor_copy(
                                    out=xcat[:, m, 0:D], in_=xn[:, t, :]
                                )
                        for k in range(KD):
                            pt = pa.tile(
                                [128, 512], tdt, tag="pa", name=f"pt{xi}_{b}_{h}_{k}"
                            )
                            for t in range(4):
                                nc.tensor.transpose(
                                    pt[:, t * 128 : (t + 1) * 128],
                                    xn[:, t, k * 128 : (k + 1) * 128],
                                    ident_f,
                                )
                            dst = xt[:, k, h * 512 : (h + 1) * 512]
                            if k % 2 == 0:
                                nc.scalar.copy(out=dst, in_=pt)
                            else:
                                nc.vector.tensor_copy(out=dst, in_=pt)
                    xts.append(xt)
                    xcats.append(xcat)
                x1t, x2t = xts
                x1cat, x2cat = xcats

                x1t_mm, x2t_mm = x1t[:], x2t[:]

                # ---- att = x1 @ x2.T per tile, fused u = exp(att - K) ----
                u = up.tile([128, MI, L], BF16, tag="u", name=f"u_{b}")
                for m in range(MI):
                    for n in range(NJ):
                        patt = pa.tile(
                            [128, 512], F32, tag="pa", name=f"patt_{b}_{m}_{n}"
                        )
                        for k in range(KD):
                            nc.tensor.matmul(
                                patt,
                                lhsT=x1t_mm[:, k, m * 128 : (m + 1) * 128],
                                rhs=x2t_mm[:, k, n * 512 : (n + 1) * 512],
                                start=(k == 0),
                                stop=(k == KD - 1),
                            )
                        nc.scalar.activation(
                            out=u[:, m, n * 512 : (n + 1) * 512],
                            in_=patt,
                            func=mybir.ActivationFunctionType.Exp,
                            bias=negk,
                            scale=1.0,
                        )

                # ---- uT (bf16) via PE transpose ----
                ut = up.tile([128, MI, L], BF16, tag="ut", name=f"ut_{b}")
                for k in range(MI):
                    ptr = pa.tile([128, L], BF16, tag="pa", name=f"ptr_{b}_{k}")
                    for m in range(MI):
                        nc.tensor.transpose(
                            ptr[:, m * 128 : (m + 1) * 128],
                            u[:, m, k * 128 : (k + 1) * 128],
                            ident_b,
                        )
                    if k % 2 == 0:
                        nc.scalar.copy(out=ut[:, k, :], in_=ptr)
                    else:
                        nc.vector.tensor_copy(out=ut[:, k, :], in_=ptr)

                # ---- out1 = u.T @ [x1|1];  out2 = uT.T @ [x2|1] ----
                for oi, (w, xc, od) in enumerate(
                    ((u, x1cat, o1d), (ut, x2cat, o2d))
                ):
                    for m in range(MI):
                        pout = po.tile(
                            [128, D + 1], F32, tag="po", name=f"pout{oi}_{b}_{m}"
                        )
                        for k in range(MI):
                            nc.tensor.matmul(
                                pout[:, 0:512],
                                lhsT=w[:, k, m * 128 : (m + 1) * 128],
                                rhs=xc[:, k, 0:512],
                                start=(k == 0),
                                stop=(k == MI - 1),
                            )
                        for k in range(MI):
                            nc.tensor.matmul(
                                pout[:, 512 : D + 1],
                                lhsT=w[:, k, m * 128 : (m + 1) * 128],
                                rhs=xc[:, k, 512 : D + 1],
                                start=(k == 0),
                                stop=(k == MI - 1),
                            )
                        r = smallp.tile([128, 1], F32, tag="r", name=f"r{oi}_{b}_{m}")
                        nc.vector.reciprocal(r, pout[:, D : D + 1])
                        o = outsp.tile([128, D], F32, tag="o", name=f"o{oi}_{b}_{m}")
                        if m % 2 == 0:
                            nc.scalar.mul(o, pout[:, 0:D], r)
                        else:
                            nc.vector.tensor_scalar_mul(o, pout[:, 0:D], r)
                        nc.sync.dma_start(
                            out=od[b, m * 128 : (m + 1) * 128, :], in_=o
                        )

    nc.compile()
    return nc


_NC = None


def _get_nc():
    global _NC
    if _NC is None:
        _NC = _build()
    return _NC


def kernel(input_1: np.ndarray, input_2: np.ndarray):
    nc = _get_nc()
    x1 = np.ascontiguousarray(np.asarray(input_1), dtype=np.float32)
    x2 = np.ascontiguousarray(np.asarray(input_2), dtype=np.float32)
    in_maps = [
        {
            "input_1": x1[i * BPC : (i + 1) * BPC],
            "input_2": x2[i * BPC : (i + 1) * BPC],
        }
        for i in range(NCORES)
    ]
    res = None
    err = None
    for _attempt in range(2):
        try:
            res = run_bass_kernel_spmd(nc, in_maps, core_ids=list(range(NCORES)))
            break
        except Exception as e:  # transient NRT/device failures: retry once
            err = e
    if res is None:
        raise err
    out1 = np.concatenate([res.results[i]["out1"] for i in range(NCORES)], axis=0)
    out2 = np.concatenate([res.results[i]["out2"] for i in range(NCORES)], axis=0)
    return (out1, out2)
